# revision 3
# baseline (speedup 1.0000x reference)
"""Trainium2 Bass kernel for BasicConvolutionBlock (sparse-conv gather + matmul
+ SyncBatchNorm + ReLU), SPMD over 8 NeuronCores, voxel-parallel sharding.

Self-contained: hardcodes all shapes from the problem spec.
"""

import sys

if "/opt/trn_rl_repo" not in sys.path:
    sys.path.insert(0, "/opt/trn_rl_repo")

import numpy as np
from einops import rearrange

import concourse.bass as bass
import concourse.mybir as mybir
import concourse.tile as tile
from concourse.bass_utils import run_bass_kernel_spmd

# ---------------- problem constants ----------------
N = 500_000
CIN = 32
COUT = 64
K = 27
KP = 28          # k padded to 28 = 7 groups of 4 (group 6 has a zero-W pad row)
GROUPS = 7
NCORES = 8
VPC = N // NCORES          # 62500 voxels per core
U = 128                    # partition dim of conv_store (voxel subindex)
BN_EPS = 1e-5

F32 = mybir.dt.float32
I32 = mybir.dt.int32


def _pick_T(vpc, tch):
    # T: blocks of 128 voxels per core; must be even (2 blocks/iter) and
    # divisible by tch (idx chunking / phase-3 chunking).
    t = -(-vpc // U)  # ceil
    # round T up to a multiple of lcm(2, tch)
    l = 2 * tch // int(np.gcd(2, tch))
    return ((t + l - 1) // l) * l


class Plan:
    """Geometry for one core's kernel (parameterized so small configs can be
    simulated quickly)."""

    def __init__(self, n_table, vpc, tch=70, dtype=F32):
        self.n_table = n_table            # rows in feats_pad (incl. zero row)
        self.vpc = vpc                    # real voxels this shard handles
        self.TCH = tch
        self.T = _pick_T(vpc, tch)        # padded block count
        self.VBAR = U * self.T            # padded voxels
        self.ITERS = self.T // 2
        self.NCHUNK = self.T // tch
        self.IT_PER_CHUNK = tch // 2
        self.dtype = dtype                # gather/matmul dtype (F32 or BF16)


def build_nc(plan: Plan, n_total, ncores=NCORES):
    """Build the SPMD Bass program (same program for all cores)."""
    p = plan
    dt = p.dtype
    nc = bass.Bass()

    feats_d = nc.declare_dram_parameter("feats_pad", [p.n_table, CIN], dt, isOutput=False)
    idx_d = nc.declare_dram_parameter("idx", [U, p.T * KP], I32, isOutput=False)
    wst_d = nc.declare_dram_parameter("wst", [128, GROUPS * COUT], dt, isOutput=False)
    gamma_d = nc.declare_dram_parameter("gamma", [COUT, 1], F32, isOutput=False)
    beta_d = nc.declare_dram_parameter("beta", [COUT, 1], F32, isOutput=False)
    ident_d = nc.declare_dram_parameter("ident64", [COUT, COUT], F32, isOutput=False)
    out_d = nc.declare_dram_parameter("out", [p.VBAR, COUT], F32, isOutput=True)

    GW = 2 * KP * CIN  # 1792: gathered elems per iter (2 blocks x 28 k x 32 cin)

    with tile.TileContext(nc) as tc:
        with (
            tc.tile_pool(name="const", bufs=1) as cpool,
            tc.tile_pool(name="store", bufs=1) as spool,
            tc.tile_pool(name="idx", bufs=2) as ipool,
            tc.tile_pool(name="gsw", bufs=3) as gpool,
            tc.tile_pool(name="g4t", bufs=2) as tpool,
            tc.tile_pool(name="s1", bufs=2) as s1pool,
            tc.tile_pool(name="misc", bufs=1) as mpool,
            tc.tile_pool(name="psA", bufs=2, space="PSUM") as psA,
            tc.tile_pool(name="ps2", bufs=2, space="PSUM") as ps2pool,
            tc.tile_pool(name="psS", bufs=1, space="PSUM") as psS,
            tc.tile_pool(name="dram", bufs=1, space="DRAM") as dpool,
        ):
            # ---- constants ----
            # PE LDWEIGHTS only supports one sync-wait slot, so anything a
            # matmul reads must have a DVE writer (waits then coalesce on the
            # DVE semaphore with the data-producer waits).
            wst_stage = cpool.tile([128, GROUPS * COUT], dt, tag="wst_stage")
            nc.sync.dma_start(out=wst_stage[:], in_=wst_d[:])
            wst_t = cpool.tile([128, GROUPS * COUT], dt, tag="wst")
            nc.vector.tensor_copy(out=wst_t[:], in_=wst_stage[:])
            id_stage = cpool.tile([COUT, COUT], F32, tag="id_stage")
            nc.sync.dma_start(out=id_stage[:], in_=ident_d[:])
            ident64 = cpool.tile([COUT, COUT], F32, tag="id64")
            nc.vector.tensor_copy(out=ident64[:], in_=id_stage[:])
            ones_col = cpool.tile([128, 1], F32, tag="ones_col")
            nc.vector.memset(ones_col[:], 1.0)
            ones_row = cpool.tile([1, 128], F32, tag="ones_row")
            nc.vector.memset(ones_row[:], 1.0)
            gamma_t = cpool.tile([COUT, 1], F32, tag="gamma")
            nc.sync.dma_start(out=gamma_t[:], in_=gamma_d[:])
            beta_t = cpool.tile([COUT, 1], F32, tag="beta")
            nc.sync.dma_start(out=beta_t[:], in_=beta_d[:])

            # conv output, SBUF-resident for the whole kernel
            conv_store = spool.tile([U, p.T, COUT], F32, tag="conv")

            # persistent stat accumulators (PSUM)
            psum_sum = psS.tile([COUT, 1], F32, tag="pssum")
            psum_sq = psS.tile([COUT, COUT], F32, tag="pssq")

            # ---- phase 1: gather / conv / stats ----
            idx_t = None
            for i in range(p.ITERS):
                c, wi = divmod(i, p.IT_PER_CHUNK)
                if wi == 0:
                    idx_t = ipool.tile([U, p.TCH * KP], I32, tag="idx")
                    nc.sync.dma_start(
                        out=idx_t[:],
                        in_=idx_d[:, c * p.TCH * KP:(c + 1) * p.TCH * KP],
                    )
                gsw = gpool.tile([U, GW], dt, tag="gsw")
                # The qPoolDynamic indirect DMA only supports one index per
                # partition per instruction (dest [128, contiguous-run]); a
                # multi-index offset AP silently misbehaves on HW. So issue
                # one 128-row gather per (block, k-slot) column.
                base = wi * 2 * KP
                for m in range(2 * KP):
                    nc.gpsimd.indirect_dma_start(
                        out=gsw[:, m * CIN:(m + 1) * CIN],
                        out_offset=None,
                        in_=feats_d[:],
                        in_offset=bass.IndirectOffsetOnAxis(
                            ap=idx_t[:, base + m:base + m + 1], axis=0
                        ),
                    )
                g4t = tpool.tile([U, 2, GROUPS, 128], dt, tag="g4t")
                g4flat = g4t[:].rearrange("p a b c -> p (a b c)")
                nc.vector.transpose(out=g4flat, in_=gsw[:])

                psumA = psA.tile([COUT, 256], F32, tag="psA")
                for g in range(GROUPS):
                    nc.tensor.matmul(
                        out=psumA[:],
                        lhsT=wst_t[:, g * COUT:(g + 1) * COUT],
                        rhs=g4t[:, :, g, :],
                        start=(g == 0),
                        stop=(g == GROUPS - 1),
                    )
                s1 = s1pool.tile([COUT, 256], F32, tag="s1")
                nc.scalar.copy(out=s1[:], in_=psumA[:])
                for blk in range(2):
                    t_idx = 2 * i + blk
                    ps2 = ps2pool.tile([128, COUT], F32, tag="ps2")
                    nc.tensor.transpose(
                        out=ps2[:],
                        in_=s1[:, blk * 128:(blk + 1) * 128],
                        identity=ident64[:],
                    )
                    cs = conv_store[:, t_idx, :]
                    nc.vector.tensor_copy(out=cs, in_=ps2[:])
                    nc.tensor.matmul(
                        out=psum_sum[:],
                        lhsT=cs,
                        rhs=ones_col[:],
                        start=(t_idx == 0),
                        stop=(t_idx == p.T - 1),
                        skip_group_check=True,
                    )
                    nc.tensor.matmul(
                        out=psum_sq[:],
                        lhsT=cs,
                        rhs=cs,
                        start=(t_idx == 0),
                        stop=(t_idx == p.T - 1),
                        skip_group_check=True,
                    )

            # ---- phase 2: stats -> collective -> scale/shift ----
            tmp64 = mpool.tile([COUT, COUT], F32, tag="tmp64")
            nc.vector.tensor_tensor(
                out=tmp64[:], in0=psum_sq[:], in1=ident64[:],
                op=mybir.AluOpType.mult,
            )
            stats2 = mpool.tile([COUT, 2], F32, tag="stats2")
            nc.vector.tensor_reduce(
                out=stats2[:, 1:2], in_=tmp64[:],
                axis=mybir.AxisListType.X, op=mybir.AluOpType.add,
            )
            nc.vector.tensor_copy(out=stats2[:, 0:1], in_=psum_sum[:])

            cc_in = dpool.tile([COUT, 2], F32, tag="ccin")
            cc_out = dpool.tile([COUT, 2], F32, tag="ccout")
            nc.sync.dma_start(out=cc_in[:], in_=stats2[:])
            nc.gpsimd.collective_compute(
                "AllReduce",
                mybir.AluOpType.add,
                replica_groups=[list(range(ncores))],
                ins=[cc_in.opt()],
                outs=[cc_out.opt()],
            )
            statsg = mpool.tile([COUT, 2], F32, tag="statsg")
            nc.sync.dma_start(out=statsg[:], in_=cc_out[:])

            inv_n = 1.0 / float(n_total)
            mean_c = mpool.tile([COUT, 1], F32, tag="mean")
            ex2_c = mpool.tile([COUT, 1], F32, tag="ex2")
            nc.vector.tensor_scalar_mul(mean_c[:], statsg[:, 0:1], inv_n)
            nc.vector.tensor_scalar_mul(ex2_c[:], statsg[:, 1:2], inv_n)
            var_c = mpool.tile([COUT, 1], F32, tag="var")
            nc.vector.tensor_tensor(
                out=var_c[:], in0=mean_c[:], in1=mean_c[:],
                op=mybir.AluOpType.mult,
            )
            nc.vector.tensor_tensor(
                out=var_c[:], in0=ex2_c[:], in1=var_c[:],
                op=mybir.AluOpType.subtract,
            )
            nc.vector.tensor_scalar_add(var_c[:], var_c[:], BN_EPS)
            std_c = mpool.tile([COUT, 1], F32, tag="std")
            nc.scalar.sqrt(out=std_c[:], in_=var_c[:])
            inv_c = mpool.tile([COUT, 1], F32, tag="inv")
            nc.vector.reciprocal(out=inv_c[:], in_=std_c[:])
            scale_c = mpool.tile([COUT, 1], F32, tag="scale")
            nc.vector.tensor_tensor(
                out=scale_c[:], in0=gamma_t[:], in1=inv_c[:],
                op=mybir.AluOpType.mult,
            )
            shift_c = mpool.tile([COUT, 1], F32, tag="shift")
            nc.vector.tensor_tensor(
                out=shift_c[:], in0=mean_c[:], in1=scale_c[:],
                op=mybir.AluOpType.mult,
            )
            nc.vector.tensor_tensor(
                out=shift_c[:], in0=beta_t[:], in1=shift_c[:],
                op=mybir.AluOpType.subtract,
            )

            # transpose [64,1] -> [1,64] then broadcast to 128 partitions
            def bcast_row(src_col, tag):
                ps_t = psA.tile([1, COUT], F32, tag="psA")
                nc.tensor.transpose(
                    out=ps_t[:], in_=src_col[:], identity=ident64[:]
                )
                row = mpool.tile([1, COUT], F32, tag=tag + "_row")
                nc.vector.tensor_copy(out=row[:], in_=ps_t[:])
                ps_b = psA.tile([128, COUT], F32, tag="psA")
                nc.tensor.matmul(
                    out=ps_b[:], lhsT=ones_row[:], rhs=row[:],
                    start=True, stop=True, skip_group_check=True,
                )
                bc = mpool.tile([128, 1, COUT], F32, tag=tag)
                nc.vector.tensor_copy(
                    out=bc[:].rearrange("p a b -> p (a b)"), in_=ps_b[:]
                )
                return bc

            scale_bc = bcast_row(scale_c, "scale_bc")
            shift_bc = bcast_row(shift_c, "shift_bc")

            # ---- phase 3: normalize + relu + store ----
            out_v = out_d[:].rearrange("(u t) o -> u (t o)", u=U)
            for c in range(p.NCHUNK):
                seg = conv_store[:, c * p.TCH:(c + 1) * p.TCH, :]
                sb = scale_bc[:].to_broadcast([U, p.TCH, COUT])
                hb = shift_bc[:].to_broadcast([U, p.TCH, COUT])
                nc.vector.tensor_tensor(
                    out=seg, in0=seg, in1=sb, op=mybir.AluOpType.mult
                )
                nc.vector.tensor_tensor(
                    out=seg, in0=seg, in1=hb, op=mybir.AluOpType.add
                )
                nc.scalar.activation(
                    out=seg, in_=seg, func=mybir.ActivationFunctionType.Relu
                )
                nc.sync.dma_start(
                    out=out_v[:, c * p.TCH * COUT:(c + 1) * p.TCH * COUT],
                    in_=seg,
                )

    import bass_rust as _bass_rust
    _bass_rust.generate_event_semaphores(nc)
    return nc


# ---------------- host-side data prep ----------------

def pack_inputs(feats, W, gamma, beta, nbr, plan: Plan, ncores=NCORES):
    """Build per-core in_maps. feats/W/... are full numpy arrays."""
    p = plan
    n = feats.shape[0]
    sentinel = n  # gathers the zero row of feats_pad

    np_dt = np.float32
    feats_pad = np.concatenate(
        [feats.astype(np_dt), np.zeros((1, CIN), np_dt)], axis=0
    )
    Wpad = np.zeros((KP, CIN, COUT), np_dt)
    Wpad[:K] = W.astype(np_dt)
    # Wst[32a + c, g, o] = Wpad[4g + a, c, o]
    wst = rearrange(Wpad, "(g a) c o -> (a c) (g o)", g=GROUPS, a=4)
    wst = np.ascontiguousarray(wst, dtype=np_dt)

    nbr32 = nbr.astype(np.int32)
    vpc = p.vpc
    in_maps = []
    for ci in range(ncores):
        nbk = np.full((KP, p.VBAR), sentinel, np.int32)
        lo, hi = ci * vpc, (ci + 1) * vpc
        nbk[:K, :vpc] = nbr32[:, lo:hi]
        # shard-local gather targets are global rows of feats_pad (replicated)
        B = nbk.reshape(KP, U, p.T)  # [k, u, t] ; global voxel v = u*T + t
        idx = rearrange(B, "(g a) (b y) t -> (a y) (t g b)",
                        g=GROUPS, a=4, b=4, y=32)
        idx = np.ascontiguousarray(idx, dtype=np.int32)
        in_maps.append({
            "feats_pad": feats_pad,
            "idx": idx,
            "wst": wst,
            "gamma": np.ascontiguousarray(
                gamma.astype(np_dt).reshape(COUT, 1)),
            "beta": np.ascontiguousarray(
                beta.astype(np_dt).reshape(COUT, 1)),
            "ident64": np.eye(COUT, dtype=np.float32),
        })
    return in_maps


def unpack_outputs(results, plan: Plan, ncores=NCORES):
    p = plan
    outs = []
    for ci in range(ncores):
        o = np.asarray(results[ci]["out"])  # [VBAR, COUT]
        # voxel v_local = u*T + t; rows >= vpc are padding
        outs.append(o[:p.vpc])
    return np.concatenate(outs, axis=0).astype(np.float32)


_CACHE = {}


def _get_compiled():
    key = "default"
    if key not in _CACHE:
        plan = Plan(N + 1, VPC)
        nc = build_nc(plan, N)
        _CACHE[key] = (plan, nc)
    return _CACHE[key]


def kernel(feats, W, gamma, beta, nbr):
    feats = np.asarray(feats)
    W = np.asarray(W)
    gamma = np.asarray(gamma)
    beta = np.asarray(beta)
    nbr = np.asarray(nbr)
    plan, nc = _get_compiled()
    in_maps = pack_inputs(feats, W, gamma, beta, nbr, plan)
    res = run_bass_kernel_spmd(nc, in_maps, core_ids=list(range(NCORES)))
    return unpack_outputs(res.results, plan)


if __name__ == "__main__":
    # smoke test with random data
    rng = np.random.default_rng(0)
    feats = rng.standard_normal((N, CIN), dtype=np.float32)
    W = rng.standard_normal((K, CIN, COUT), dtype=np.float32) * 0.05
    gamma = np.ones(COUT, np.float32)
    beta = np.zeros(COUT, np.float32)
    nbr = rng.integers(0, N + 1, size=(K, N), dtype=np.int64)
    out = kernel(feats, W, gamma, beta, nbr)
    print(out.shape, out.dtype)
